# revision 1
# baseline (speedup 1.0000x reference)
"""Correlation kernel (max_disp=1, 9 offsets) for Trainium2, 8 NeuronCores.

Computation (per batch b):
    out[dx*3+dy, i, j] = mean_c( x1[c,i,j] * pad(x2)[c, i+dy, j+dx] )
with B=8, C=512, H=W=96, pad=1 on each spatial side.

Sharding: data-parallel over batch — core b handles batch b. No collectives.

Per-core strategy:
  - C (512) on SBUF partitions, 4 chunks of 128; spatial (96x96=9216) on free dim.
  - Inputs DMA'd with fp32->bf16 cast (SWDGE). x2 lands in a zero-padded
    [128, 98, 98] tile; a second copy shifted by one column (SBUF->SBUF DMA)
    keeps all 9 shifted views 4B-aligned so DVE tensor_mul runs in 2x mode.
  - VectorE: 9 offsets x 4 chunks bf16 elementwise products [128, 9216].
  - TensorE: partition-dim reduction via matmuls whose stationary operand is a
    sliding one-hot-column-of-ones [128, M] slice; offset k, 512-col block bb
    accumulates into row 18*g+bb of a persistent PSUM tile ([126,512] holds
    offsets 0-6, [36,512] holds offsets 7-8), accumulating all 4 C-chunks.
  - VectorE scales PSUM by 1/512 into SBUF; HWDGE DMAs reshape to [9,96,96].
"""

import os
import sys

for _p in ("/opt/trn_rl_repo",):
    if os.path.isdir(_p) and _p not in sys.path:
        sys.path.insert(0, _p)

from contextlib import ExitStack

import numpy as np

import concourse.bass as bass
import concourse.mybir as mybir
import concourse.tile as tile
from concourse import bacc
from concourse.bass_utils import run_bass_kernel_spmd

B, C, H, W = 8, 512, 96, 96
NCORES = 8
PW = W + 2          # padded spatial width
HW = H * W          # 9216 free elems
NCHUNK = C // 128   # 4
NBLK = HW // 512    # 18 512-col blocks
F32 = mybir.dt.float32
BF16 = mybir.dt.bfloat16


def _corr_body(ctx: ExitStack, tc: "tile.TileContext", out_t, x1_t, x2_t, nchunk=NCHUNK):
    nc = tc.nc

    wpool = ctx.enter_context(tc.tile_pool(name="wm", bufs=1))
    x1pool = ctx.enter_context(tc.tile_pool(name="x1", bufs=2))
    evpool = ctx.enter_context(tc.tile_pool(name="ev", bufs=2))
    odpool = ctx.enter_context(tc.tile_pool(name="od", bufs=2))
    prpool = ctx.enter_context(tc.tile_pool(name="pr", bufs=int(os.environ.get("CORR_PROD_BUFS", "4"))))
    pspool = ctx.enter_context(
        tc.tile_pool(name="ps", bufs=1, space=bass.MemorySpace.PSUM)
    )
    outpool = ctx.enter_context(tc.tile_pool(name="ot", bufs=1))

    # Sliding ones-column masters: slice wm*[:, s : s+M] has its all-ones column
    # at local position m0 when s = ones_col - m0. Two masters (ones at col 126
    # and 127) keep s even for either parity of m0, so every LDWEIGHTS source is
    # 4-byte aligned (bf16).
    wmE = wpool.tile([128, 256], BF16)
    nc.vector.memset(wmE[:, :], 0.0)
    nc.vector.memset(wmE[:, 126:127], 1.0)
    wmO = wpool.tile([128, 256], BF16)
    nc.vector.memset(wmO[:, :], 0.0)
    nc.vector.memset(wmO[:, 127:128], 1.0)

    def wslice(m0: int, M: int):
        wm, col = (wmE, 126) if m0 % 2 == 0 else (wmO, 127)
        s = col - m0
        return wm[:, s : s + M]

    # Persistent PSUM accumulators: offsets 0..6 -> rows 18k+bb of [126,512];
    # offsets 7..8 -> rows 18*(k-7)+bb of [36,512].
    psA = pspool.tile([126, 512], F32)
    psB = pspool.tile([36, 512], F32)

    x1f = x1_t.ap()  # [512, 96, 96] f32 DRAM
    x2f = x2_t.ap()

    started = {"A": False, "B": False}

    for ch in range(nchunk):
        p0 = ch * 128
        x1bf = x1pool.tile([128, H, W], BF16)
        nc.gpsimd.dma_start(out=x1bf[:, :, :], in_=x1f[p0 : p0 + 128, :, :])

        ev = evpool.tile([128, PW, PW], BF16)
        # zero borders (rows 0,97; cols 0,97 of rows 1..96)
        nc.vector.memset(ev[:, 0, :], 0.0)
        nc.vector.memset(ev[:, PW - 1, :], 0.0)
        nc.vector.memset(ev[:, 1 : PW - 1, 0], 0.0)
        nc.vector.memset(ev[:, 1 : PW - 1, PW - 1], 0.0)
        nc.gpsimd.dma_start(
            out=ev[:, 1 : PW - 1, 1 : PW - 1], in_=x2f[p0 : p0 + 128, :, :]
        )

        # odd copy: flat shift-by-one so dx=1 views are 4B-aligned.
        od = odpool.tile([128, PW, PW], BF16)
        ev_flat = ev[:, :, :].rearrange("p a b -> p (a b)")
        od_flat = od[:, :, :].rearrange("p a b -> p (a b)")
        nc.sync.dma_start(out=od_flat[:, 0 : PW * PW - 1], in_=ev_flat[:, 1 : PW * PW])

        for dx in range(3):
            for dy in range(3):
                k = dx * 3 + dy
                if dx == 1:
                    src, dxx = od, 0
                else:
                    src, dxx = ev, dx
                view = src[:, dy : dy + H, dxx : dxx + W]
                prod = prpool.tile([128, H, W], BF16)
                nc.vector.tensor_mul(prod[:, :, :], x1bf[:, :, :], view)

                if os.environ.get("CORR_SKIP_MM"):
                    continue
                prod_flat = prod[:, :, :].rearrange("p a b -> p (a b)")
                if k < 7:
                    ps, M, g, bank = psA, 126, k, "A"
                else:
                    ps, M, g, bank = psB, 36, k - 7, "B"
                for bb in range(NBLK):
                    m0 = 18 * g + bb
                    st = not started[bank]
                    started[bank] = True
                    last = (
                        ch == nchunk - 1
                        and bb == NBLK - 1
                        and ((bank == "A" and k == 6) or (bank == "B" and k == 8))
                    )
                    nc.tensor.matmul(
                        ps[:, :],
                        wslice(m0, M),
                        prod_flat[:, bb * 512 : (bb + 1) * 512],
                        start=st,
                        stop=last,
                    )

    outA = outpool.tile([126, 512], F32)
    outB = outpool.tile([36, 512], F32)
    nc.vector.tensor_scalar_mul(outA[:, :], psA[:, :], 1.0 / (128 * nchunk))
    nc.vector.tensor_scalar_mul(outB[:, :], psB[:, :], 1.0 / (128 * nchunk))

    outf = out_t.ap()  # [9, 96, 96] f32 DRAM
    for k in range(9):
        if k < 7:
            src = outA[18 * k : 18 * (k + 1), :]
        else:
            src = outB[18 * (k - 7) : 18 * (k - 6), :]
        nc.sync.dma_start(out=outf[k, :, :], in_=src)


_CACHE = {}


def _build(c=C, debug=False):
    key = ("nc", c)
    if key in _CACHE:
        return _CACHE[key]
    nchunk = c // 128
    nc = bacc.Bacc("TRN2", target_bir_lowering=False, debug=debug)
    x1_t = nc.dram_tensor("x_1", [c, H, W], F32, kind="ExternalInput")
    x2_t = nc.dram_tensor("x_2", [c, H, W], F32, kind="ExternalInput")
    out_t = nc.dram_tensor("out", [9, H, W], F32, kind="ExternalOutput")
    with tile.TileContext(nc) as tc, ExitStack() as ctx:
        _corr_body(ctx, tc, out_t, x1_t, x2_t, nchunk=nchunk)
    nc.compile()
    _CACHE[key] = nc
    return nc


def kernel(x_1: np.ndarray, x_2: np.ndarray) -> np.ndarray:
    x_1 = np.ascontiguousarray(np.asarray(x_1), dtype=np.float32)
    x_2 = np.ascontiguousarray(np.asarray(x_2), dtype=np.float32)
    assert x_1.shape == (B, C, H, W) and x_2.shape == (B, C, H, W)
    nc = _build()
    in_maps = [
        {"x_1": x_1[i].copy(), "x_2": x_2[i].copy()} for i in range(NCORES)
    ]
    last_err = None
    for attempt in range(3):
        try:
            res = run_bass_kernel_spmd(nc, in_maps, list(range(NCORES)))
            out = np.stack([res.results[i]["out"] for i in range(NCORES)], axis=0)
            return out.astype(np.float32)
        except Exception as e:  # rare transient device faults — retry
            last_err = e
            import time as _time

            _time.sleep(5.0 * (attempt + 1))
    raise last_err


if __name__ == "__main__":
    rng = np.random.default_rng(0)
    a = rng.standard_normal((B, C, H, W), dtype=np.float32)
    b = rng.standard_normal((B, C, H, W), dtype=np.float32)
    o = kernel(a, b)
    print("out", o.shape, o.dtype, float(np.abs(o).max()))

